# revision 3
# baseline (speedup 1.0000x reference)
"""Distributed Trainium2 Bass kernel for causal multi-head attention with RoPE.

Problem: B=2, T=2048, C=1024, H=16 heads, D=64. 8 NeuronCores.

Sharding (2x4 grid): core c handles batch b = c//4 and the 4 heads
g = c%4 -> heads [4g..4g+4). QKV projections + RoPE + causal attention run
fully locally per core in a "transposed" layout (qT/kT = [D_heads, T]):

  qT = Wq_slice.T @ x.T            (lhsT = Wq natural, rhs = x.T)
  scoresT[k,q] = kT.T-block @ qT   (softmax along PARTITION axis)
  avT = [v|1].T @ exp(scoresT)     (ones column yields softmax denominators)
  outW = Wo_cols.T @ attn_allT     (attn stays transposed through Wo)

Schedule: QKV/RoPE chunk A(nch) and attention q-chunk B(qc=nch) are
interleaved so the tensor engine never idles (HAM stays at 2.4GHz) and the
scalar engine's exp work spreads over the whole kernel. Within B, scores
run 2 k-blocks ahead of the AV matmuls (software pipeline) so the PE never
waits on exp. exp runs mostly on the scalar engine; a subset of blocks use
a Schraudolph-style exp on the vector engine (x*a+b written through the
fp32->int16 convert, bitcast to bf16). The causal mask multiply runs on the
otherwise-idle gpsimd engine. Softmax normalization broadcasts 1/denom via
a col-tiled ones matmul + reciprocal_approx on [128,512] (parallel lanes).
Attention output (pre-Wo, bf16) is AllGathered in each 4-core group; each
core computes its 256-row slice of the transposed Wo projection. Output is
bf16 on device, cast to fp32 on host.
"""

import numpy as np
import ml_dtypes

import concourse.bacc as bacc
import concourse.mybir as mybir
import concourse.tile as tile
from concourse.bass_utils import run_bass_kernel_spmd

B, T, C, H, D = 2, 2048, 1024, 16, 64
NCORES = 8
HPC = 4              # heads per core
CPC = HPC * D        # channels per core (256)
NPAIR = 2            # head pairs per core
QC = 4               # q-chunks of 512
KB = T // 128        # k-blocks of 128
CCH = C // 128       # contraction chunks of 128
F32 = mybir.dt.float32
BF16 = mybir.dt.bfloat16
I16 = mybir.dt.int16
AF = mybir.ActivationFunctionType
ALU = mybir.AluOpType
RGROUPS = [[0, 1, 2, 3], [4, 5, 6, 7]]

# Schraudolph exp on DVE: exp(0.125*x) ~= bf16_bits(int16(x*EXP_A + EXP_B))
EXP_A = 23.08312065
EXP_B = 16250.5
# (qc, kb) blocks (non-diagonal) whose exp runs on DVE instead of ACT
DVE_KBS = {(1, 1), (2, 1), (2, 5), (3, 1), (3, 5), (3, 9)}

_cache = {}


def _build_nc():
    nc = bacc.Bacc(None, target_bir_lowering=False, debug=False, num_devices=NCORES)

    xT = nc.declare_dram_parameter("xT", [C, T], BF16, isOutput=False)
    wq = nc.declare_dram_parameter("wq", [C, CPC], BF16, isOutput=False)
    wk = nc.declare_dram_parameter("wk", [C, CPC], BF16, isOutput=False)
    wv = nc.declare_dram_parameter("wv", [C, CPC], BF16, isOutput=False)
    wo = nc.declare_dram_parameter("wo", [C, CPC], BF16, isOutput=False)
    cosP = nc.declare_dram_parameter("cosP", [128, T], BF16, isOutput=False)
    sinP = nc.declare_dram_parameter("sinP", [128, T], BF16, isOutput=False)
    maskut = nc.declare_dram_parameter("maskut", [128, 256], BF16, isOutput=False)
    smat = nc.declare_dram_parameter("smat", [128, 128], BF16, isOutput=False)
    out = nc.declare_dram_parameter("out", [CPC, T], BF16, isOutput=True)

    with tile.TileContext(nc) as tc:
        with (
            tc.tile_pool(name="resident", bufs=1) as rp,
            tc.tile_pool(name="rope", bufs=3) as ropep,
            tc.tile_pool(name="expp", bufs=4) as expp,
            tc.tile_pool(name="outb", bufs=3) as outbp,
            tc.tile_pool(name="agsb", bufs=16) as agp,
            tc.tile_pool(name="small", bufs=2) as smp,
            tc.tile_pool(name="ps", bufs=3, space="PSUM") as psp,
            tc.tile_pool(name="pav", bufs=1, space="PSUM") as pav,
            tc.tile_pool(name="dram", bufs=1, space="DRAM") as dram,
        ):
            # ---------------- resident SBUF ----------------
            xbf = rp.tile([128, CCH * T], BF16)          # x.T in [nch][cc] blocks
            wqbf = rp.tile([128, CCH * CPC], BF16)
            wkbf = rp.tile([128, CCH * CPC], BF16)
            wvbf = rp.tile([128, CCH * CPC], BF16)
            wobf = rp.tile([128, CCH * CPC], BF16)
            cos_sb = rp.tile([128, T], BF16)
            sin_sb = rp.tile([128, T], BF16)
            mask_bf = rp.tile([128, 256], BF16)
            smat_bf = rp.tile([128, 128], BF16)
            ones_sb = rp.tile([1, 64], BF16)
            qTbf = rp.tile([128, NPAIR * T], BF16)       # rope'd qT, per pair
            kTbf = rp.tile([128, NPAIR * T], BF16)
            vsb = rp.tile([128, HPC * KB * 65], BF16)    # [v | 1] per head per k-block

            qeng = [nc.sync, nc.scalar, nc.gpsimd]

            def load_x(nch, engs):
                for cc in range(CCH):
                    engs[cc % len(engs)].dma_start(
                        xbf[:, (nch * CCH + cc) * 512:(nch * CCH + cc + 1) * 512],
                        xT[cc * 128:(cc + 1) * 128, nch * 512:(nch + 1) * 512])

            # stage 0: what A(0) needs first: wq, wk, x(nch0)
            for cc in range(CCH):
                qeng[cc % 3].dma_start(
                    wqbf[:, cc * CPC:(cc + 1) * CPC], wq[cc * 128:(cc + 1) * 128, :])
                qeng[(cc + 1) % 3].dma_start(
                    wkbf[:, cc * CPC:(cc + 1) * CPC], wk[cc * 128:(cc + 1) * 128, :])
                qeng[(cc + 2) % 3].dma_start(
                    xbf[:, cc * 512:(cc + 1) * 512], xT[cc * 128:(cc + 1) * 128, 0:512])
            # stage 1: rope tables + wv + masks
            nc.sync.dma_start(cos_sb[:], cosP[:])
            nc.gpsimd.dma_start(sin_sb[:], sinP[:])
            for cc in range(CCH):
                qeng[cc % 2].dma_start(
                    wvbf[:, cc * CPC:(cc + 1) * CPC], wv[cc * 128:(cc + 1) * 128, :])
            nc.sync.dma_start(mask_bf[:], maskut[:])
            nc.sync.dma_start(smat_bf[:], smat[:])
            nc.gpsimd.memset(ones_sb[:], 1.0)
            nc.gpsimd.memset(vsb[:], 1.0)

            # ---------------- PSUM tag rotation (2 banks: avA / avB) ---------
            _rot = {"i": 0}

            def av_tag():
                t = "avA" if _rot["i"] % 2 == 0 else "avB"
                _rot["i"] += 1
                return t

            # ---------------- phase A: QKV + RoPE for one nch ----------------
            def emit_A(nch):
                nsl = slice(nch * 512, nch * 512 + 512)
                for wi, (w_sb, t_sb) in enumerate(((wqbf, qTbf), (wkbf, kTbf))):
                    for p in range(NPAIR):
                        ps_t = pav.tile([128, 512], F32, tag=av_tag(),
                                        name=f"pst{nch}_{wi}_{p}")
                        for cc in range(CCH):
                            nc.tensor.matmul(
                                ps_t[:],
                                w_sb[:, cc * CPC + p * 128: cc * CPC + (p + 1) * 128],
                                xbf[:, (nch * CCH + cc) * 512:(nch * CCH + cc + 1) * 512],
                                start=(cc == 0), stop=(cc == CCH - 1),
                            )
                        # RoPE: out = ps*cos + (S.T@ps)*sin'
                        qub = ropep.tile([128, 512], BF16, tag="qub")
                        if wi == 0:
                            nc.scalar.copy(qub[:], ps_t[:])
                        else:
                            nc.vector.tensor_copy(qub[:], ps_t[:])
                        rot = psp.tile([128, 1024], F32, tag="s",
                                       name=f"rot{nch}_{wi}_{p}")
                        nc.tensor.matmul(rot[:, 0:512], smat_bf[:], qub[:],
                                         start=True, stop=True)
                        t1 = ropep.tile([128, 512], F32, tag="t1")
                        nc.vector.tensor_mul(t1[:], ps_t[:], cos_sb[:, nsl])
                        t2 = ropep.tile([128, 512], F32, tag="t2")
                        nc.vector.tensor_mul(t2[:], rot[:, 0:512], sin_sb[:, nsl])
                        nc.vector.tensor_add(
                            t_sb[:, p * T + nch * 512: p * T + nch * 512 + 512],
                            t1[:], t2[:])
                # v natural layout -> vsb [128, h*KB*65 + kb*65 + (0..64)]
                for tch in range(4 * nch, 4 * nch + 4):
                    ps_v = pav.tile([128, 512], F32, tag=av_tag(), name=f"psv{tch}")
                    for cc in range(CCH):
                        nc.tensor.matmul(
                            ps_v[:, 0:CPC],
                            xbf[:, (nch * CCH + cc) * 512 + (tch % 4) * 128:
                                (nch * CCH + cc) * 512 + (tch % 4) * 128 + 128],
                            wvbf[:, cc * CPC:(cc + 1) * CPC],
                            start=(cc == 0), stop=(cc == CCH - 1),
                        )
                    # one strided DVE copy: [128, (4 heads, stride KB*65), 1, 64]
                    src = ps_v[:, 0:CPC].rearrange("p (h o d) -> p h o d", h=HPC, o=1)
                    dstv = vsb[:].rearrange("p (h kb e) -> p h kb e", h=HPC, kb=KB)[
                        :, :, tch:tch + 1, 0:64]
                    nc.vector.tensor_copy(dstv, src)

            # ---------------- phase B: attention q-chunk (both pairs) --------
            bands = [dram.tile([CPC, 512], BF16, tag=f"agin{i}", name=f"band{i}")
                     for i in range(QC - 1)]
            bands3 = [dram.tile([128, 512], BF16, tag=f"agin3{p}", name=f"band3{p}")
                      for p in range(NPAIR)]
            ag_outs = []

            def emit_flush(qc, p, avh):
                den = smp.tile([1, 1024], BF16, tag="den", name=f"den{qc}_{p}")
                nc.scalar.copy(den[:, 0:512], avh[0][64:65, :])
                nc.scalar.copy(den[:, 512:1024], avh[1][64:65, :])
                bc = psp.tile([128, 1024], F32, tag="s", name=f"bc{qc}_{p}")
                for i in range(2):
                    nc.tensor.matmul(bc[i * 64:(i + 1) * 64, 0:512], ones_sb[:],
                                     den[:, i * 512:(i + 1) * 512],
                                     start=True, stop=True, tile_position=(0, i * 64))
                recb = smp.tile([128, 512], F32, tag="recb", name=f"recb{qc}_{p}")
                nc.vector.reciprocal_approx_fast(recb[:], bc[:, 0:512])
                ob = outbp.tile([128, 512], BF16, tag="ob", name=f"ob{qc}_{p}")
                for i in range(2):
                    nc.vector.tensor_mul(ob[i * 64:(i + 1) * 64, :],
                                         avh[i][0:64, :], recb[i * 64:(i + 1) * 64, :])
                if qc == 3:
                    nc.sync.dma_start(bands3[p][:], ob[:])
                    ag_out = dram.tile([4 * 128, 512], BF16, tag=f"agout3{p}",
                                       name=f"agout3{p}")
                    nc.gpsimd.collective_compute(
                        "AllGather", mybir.AluOpType.bypass,
                        replica_groups=RGROUPS,
                        ins=[bands3[p].opt()], outs=[ag_out.opt()],
                    )
                    ag_outs.append(ag_out)
                else:
                    nc.sync.dma_start(bands[qc][p * 128:(p + 1) * 128, :], ob[:])
                    if p == NPAIR - 1:
                        ag_out = dram.tile([4 * CPC, 512], BF16, tag=f"agout{qc}",
                                           name=f"agout{qc}")
                        nc.gpsimd.collective_compute(
                            "AllGather", mybir.AluOpType.bypass,
                            replica_groups=RGROUPS,
                            ins=[bands[qc].opt()], outs=[ag_out.opt()],
                        )
                        ag_outs.append(ag_out)

            def emit_B(qc):
                kmax = 4 * qc + 4
                pend = []   # deferred AV emitters, depth-2 pipeline

                def drain(n):
                    while len(pend) > n:
                        pend.pop(0)()

                for p in range(NPAIR):
                    avh = [pav.tile([65, 512], F32, tag=av_tag(),
                                    name=f"av{qc}_{p}_{i}") for i in range(2)]
                    for kb in range(kmax):
                        nqs = max(qc * 512, kb * 128)
                        noff = nqs - qc * 512
                        n = 512 - noff
                        diag = (nqs == kb * 128)
                        ps_s = psp.tile([128, 1024], F32, tag="s",
                                        name=f"pss{qc}_{p}_{kb}")
                        for i in range(2):
                            hs = slice(i * 64, (i + 1) * 64)
                            nc.tensor.matmul(
                                ps_s[:, i * 512: i * 512 + n],
                                kTbf[hs, p * T + kb * 128: p * T + kb * 128 + 128],
                                qTbf[hs, p * T + nqs: p * T + nqs + n],
                                start=True, stop=True,
                                tile_position=(i * 64, 0),
                            )
                        e = expp.tile([128, 1024], BF16, tag="e",
                                      name=f"e{qc}_{p}_{kb}")
                        if (qc, kb) in DVE_KBS:
                            nc.vector.tensor_scalar(
                                e[:, 0:512 + n].bitcast(I16), ps_s[:, 0:512 + n],
                                EXP_A, EXP_B, ALU.mult, ALU.add)
                        else:
                            nc.scalar.activation(e[:, 0:512 + n], ps_s[:, 0:512 + n],
                                                 AF.Exp, scale=0.125)
                        if diag:
                            ev = e[:].rearrange("p (b c) -> p b c", b=2)[:, :, 0:128]
                            mv = mask_bf[:].rearrange("p (b c) -> p b c", b=2)
                            nc.gpsimd.tensor_tensor(ev, ev, mv, ALU.mult)

                        def mk_av(p=p, kb=kb, noff=noff, n=n, e=e, avh=avh):
                            def go():
                                for i in range(2):
                                    h = 2 * p + i
                                    vbase = h * KB * 65 + kb * 65
                                    nc.tensor.matmul(
                                        avh[i][:, noff:512],
                                        vsb[:, vbase: vbase + 65],
                                        e[:, i * 512: i * 512 + n],
                                        start=(kb == 0), stop=(kb == kmax - 1),
                                    )
                                if kb == kmax - 1:
                                    emit_flush(qc, p, avh)
                            return go

                        pend.append(mk_av())
                        drain(2)
                drain(0)

            # ---------------- phase C: Wo per q-chunk (transposed output) ----
            _ag_sb = {}

            def wo_srcs(qc):
                if qc == 3:
                    order = [0, 2, 4, 6, 1, 3, 5, 7]
                    srcs = {cc: (ag_outs[3 + cc % 2], (cc // 2) * 128)
                            for cc in range(CCH)}
                else:
                    order = list(range(CCH))
                    srcs = {cc: (ag_outs[qc], cc * 128) for cc in range(CCH)}
                return order, srcs

            def emit_wo_loads(qc, half=None):
                order, srcs = wo_srcs(qc)
                for cc in order:
                    if half is not None and cc % 2 != half:
                        continue
                    src, row = srcs[cc]
                    t = agp.tile([128, 512], BF16, name=f"ag_{qc}_{cc}", tag="ag")
                    nc.sync.dma_start(t[:], src[row:row + 128, :])
                    _ag_sb[(qc, cc)] = t

            def emit_wo_mms(qc):
                order, srcs = wo_srcs(qc)
                for mch in range(2):
                    ps_o = pav.tile([128, 512], F32, tag=av_tag(),
                                    name=f"pso{qc}_{mch}")
                    for idx, cc in enumerate(order):
                        nc.tensor.matmul(
                            ps_o[:],
                            wobf[:, cc * CPC + mch * 128: cc * CPC + (mch + 1) * 128],
                            _ag_sb[(qc, cc)][:],
                            start=(idx == 0), stop=(idx == CCH - 1),
                        )
                    osb = outbp.tile([128, 512], BF16, tag="osb")
                    nc.scalar.copy(osb[:], ps_o[:])
                    nc.sync.dma_start(
                        out[mch * 128:(mch + 1) * 128, qc * 512:(qc + 1) * 512], osb[:])

            # ---------------- schedule ----------------
            emit_A(0)
            load_x(1, [nc.sync, nc.gpsimd])
            for cc in range(CCH):
                qeng[cc % 2 * 2].dma_start(
                    wobf[:, cc * CPC:(cc + 1) * CPC], wo[cc * 128:(cc + 1) * 128, :])
            emit_B(0)
            emit_wo_loads(0)
            load_x(2, [nc.sync, nc.gpsimd])
            emit_A(1)
            emit_B(1)
            emit_wo_loads(1)
            load_x(3, [nc.sync, nc.gpsimd])
            emit_A(2)
            emit_wo_mms(0)
            emit_B(2)
            emit_wo_loads(2)
            emit_A(3)
            emit_wo_mms(1)
            emit_B(3)
            emit_wo_loads(3)
            emit_wo_mms(2)
            emit_wo_mms(3)
    return nc


def _get_nc():
    if "nc" not in _cache:
        nc = _build_nc()
        nc.finalize()
        _cache["nc"] = nc
    return _cache["nc"]


def _host_tables(freqs_cos, freqs_sin):
    cosP = np.empty((128, T), np.float32)
    sinP = np.empty((128, T), np.float32)
    for r in range(128):
        i = (r % 64) // 2
        cosP[r] = freqs_cos[:, i]
        sinP[r] = freqs_sin[:, i]
    maskut = np.tile(np.triu(np.ones((128, 128), np.float32)), (1, 2))
    smat = np.zeros((128, 128), np.float32)
    for i in range(64):
        smat[2 * i + 1, 2 * i] = -1.0   # rot[2i] = -q[2i+1]
        smat[2 * i, 2 * i + 1] = 1.0    # rot[2i+1] = +q[2i]
    return cosP, sinP, maskut, smat


def _install_trace_hooks():
    import sys, types
    try:
        import antenv.axon_hooks  # noqa: F401
        return True
    except ImportError:
        pass
    try:
        from trn_agent_boot.trn_boot import _ntff_profile_via_ctypes
        mod = types.ModuleType("antenv.axon_hooks")
        mod._hook = _ntff_profile_via_ctypes("/opt/axon/libaxon_pjrt.so")
        mod.set_axon_ntff_profile_hook = lambda h: setattr(mod, "_hook", h)
        mod.get_axon_ntff_profile_hook = lambda: mod._hook
        sys.modules["antenv.axon_hooks"] = mod
        import antenv
        antenv.axon_hooks = mod
        import concourse.bass_utils as bu
        bu.upload_artifacts = lambda tmpdir: f"file://{tmpdir}"
        return True
    except Exception:
        return False


def _bf16(a):
    return np.ascontiguousarray(a).astype(ml_dtypes.bfloat16)


def kernel(x, freqs_cos, freqs_sin, Wq, Wk, Wv, Wo, _trace=False):
    x = np.asarray(x, np.float32)
    freqs_cos = np.asarray(freqs_cos, np.float32)
    freqs_sin = np.asarray(freqs_sin, np.float32)
    Wq, Wk, Wv, Wo = (np.asarray(w, np.float32) for w in (Wq, Wk, Wv, Wo))
    cosP, sinP, maskut, smat = _host_tables(freqs_cos, freqs_sin)

    in_maps = []
    for c in range(NCORES):
        b, g = c // 4, c % 4
        sl = slice(g * CPC, (g + 1) * CPC)
        in_maps.append({
            "xT": _bf16(x[b].T),
            "wq": _bf16(Wq[:, sl]),
            "wk": _bf16(Wk[:, sl]),
            "wv": _bf16(Wv[:, sl]),
            "wo": _bf16(Wo[:, sl]),
            "cosP": _bf16(cosP), "sinP": _bf16(sinP),
            "maskut": _bf16(maskut), "smat": _bf16(smat),
        })

    nc = _get_nc()
    if _trace:
        _trace = _install_trace_hooks()
    res = run_bass_kernel_spmd(nc, in_maps, core_ids=list(range(NCORES)), trace=_trace)
    _cache["last_res"] = res

    out = np.empty((B, T, C), np.float32)
    for c in range(NCORES):
        b, g = c // 4, c % 4
        out[b][:, g * CPC:(g + 1) * CPC] = res.results[c]["out"].T.astype(np.float32)
    return out


# revision 8
# speedup vs baseline: 1.0477x; 1.0477x over previous
"""Distributed Trainium2 Bass kernel for causal multi-head attention with RoPE.

Problem: B=2, T=2048, C=1024, H=16 heads, D=64. 8 NeuronCores.

Sharding (2x4 grid): core c handles batch b = c//4 and the 4 heads
g = c%4 -> heads [4g..4g+4). QKV projections + RoPE + causal attention run
fully locally per core in a "transposed" layout (qT/kT = [D_heads, T]):

  qT = Wq_slice.T @ x.T            (lhsT = Wq natural, rhs = x.T)
  scoresT[k,q] = kT.T-block @ qT   (softmax along PARTITION axis)
  avT = [v|1].T @ exp(scoresT)     (ones column yields softmax denominators)
  outW = Wo_cols.T @ attn_allT     (attn stays transposed through Wo)

Schedule: QKV/RoPE chunk A(nch) and attention q-chunk B(qc) interleave as
A0 B0 A1 B1 A2 A3 B3 B2 so the tensor engine never idles (HAM stays at
2.4GHz), exp spreads over the kernel, and the LAST AllGather (qc=2 pair 1,
128 rows) is small. Within B, scores run 2 k-blocks ahead of the AV
matmuls. exp runs mostly on the scalar engine; tail-heavy blocks use a
Schraudolph exp on the vector engine (x*a+b through the fp32->int16
convert, bitcast to bf16). The causal mask multiply runs on gpsimd, which
also computes the q*cos rope product from the bf16 copy so PSUM frees after
one scalar-engine copy. Softmax 1/denom broadcasts via one [2,128]
selector matmul + reciprocal_approx on [128,512]. A dummy AllGather at
start absorbs the ~11us CC-core boot. Output is bf16, cast on host.
"""

import numpy as np
import ml_dtypes

import concourse.bacc as bacc
import concourse.mybir as mybir
import concourse.tile as tile
from concourse.bass_utils import run_bass_kernel_spmd

B, T, C, H, D = 2, 2048, 1024, 16, 64
NCORES = 8
HPC = 4              # heads per core
CPC = HPC * D        # channels per core (256)
NPAIR = 2            # head pairs per core
QC = 4               # q-chunks of 512
KB = T // 128        # k-blocks of 128
CCH = C // 128       # contraction chunks of 128
F32 = mybir.dt.float32
BF16 = mybir.dt.bfloat16
I16 = mybir.dt.int16
AF = mybir.ActivationFunctionType
ALU = mybir.AluOpType
RGROUPS = [[0, 1, 2, 3], [4, 5, 6, 7]]

# Schraudolph exp on DVE: exp(0.125*x) ~= bf16_bits(int16(x*EXP_A + EXP_B))
EXP_A = 23.08312065
EXP_B = 16250.5
# (qc, kb) non-diagonal blocks whose exp runs on DVE instead of ACT.
# B3/B2 run at the tail with no A-phase to absorb ACT load -> offload there.
DVE_KBS = {(3, 1), (3, 3), (3, 5), (3, 7), (3, 9), (3, 11),
           (2, 1), (2, 3), (2, 5), (2, 7)}

_cache = {}


def _build_nc():
    nc = bacc.Bacc(None, target_bir_lowering=False, debug=False, num_devices=NCORES)

    xT = nc.declare_dram_parameter("xT", [C, T], BF16, isOutput=False)
    wq = nc.declare_dram_parameter("wq", [C, CPC], BF16, isOutput=False)
    wk = nc.declare_dram_parameter("wk", [C, CPC], BF16, isOutput=False)
    wv = nc.declare_dram_parameter("wv", [C, CPC], BF16, isOutput=False)
    wo = nc.declare_dram_parameter("wo", [C, CPC], BF16, isOutput=False)
    cosP = nc.declare_dram_parameter("cosP", [128, T], BF16, isOutput=False)
    sinP = nc.declare_dram_parameter("sinP", [128, T], BF16, isOutput=False)
    maskut = nc.declare_dram_parameter("maskut", [128, 256], BF16, isOutput=False)
    smat = nc.declare_dram_parameter("smat", [128, 128], BF16, isOutput=False)
    out = nc.declare_dram_parameter("out", [CPC, T], BF16, isOutput=True)

    with tile.TileContext(nc) as tc:
        with (
            tc.tile_pool(name="resident", bufs=1) as rp,
            tc.tile_pool(name="rope", bufs=3) as ropep,
            tc.tile_pool(name="expp", bufs=4) as expp,
            tc.tile_pool(name="outb", bufs=3) as outbp,
            tc.tile_pool(name="agsb", bufs=16) as agp,
            tc.tile_pool(name="small", bufs=2) as smp,
            tc.tile_pool(name="ps", bufs=3, space="PSUM") as psp,
            tc.tile_pool(name="pav", bufs=1, space="PSUM") as pav,
            tc.tile_pool(name="dram", bufs=1, space="DRAM") as dram,
        ):
            # ---------------- resident SBUF ----------------
            xbf = rp.tile([128, CCH * T], BF16)          # x.T in [nch][cc] blocks
            wqbf = rp.tile([128, CCH * CPC], BF16)
            wkbf = rp.tile([128, CCH * CPC], BF16)
            wvbf = rp.tile([128, CCH * CPC], BF16)
            wobf = rp.tile([128, CCH * CPC], BF16)
            cos_sb = rp.tile([128, T], BF16)
            sin_sb = rp.tile([128, T], BF16)
            mask_bf = rp.tile([128, 256], BF16)
            smat_bf = rp.tile([128, 128], BF16)
            ones_sb = rp.tile([1, 64], BF16)
            qTbf = rp.tile([128, NPAIR * T], BF16)       # rope'd qT, per pair
            kTbf = rp.tile([128, NPAIR * T], BF16)
            vsb = rp.tile([128, HPC * KB * 65], BF16)    # [v | 1] per head per k-block

            q3 = [nc.sync, nc.scalar, nc.gpsimd]

            def load_x(nch, engs):
                for cc in range(CCH):
                    engs[cc % len(engs)].dma_start(
                        xbf[:, (nch * CCH + cc) * 512:(nch * CCH + cc + 1) * 512],
                        xT[cc * 128:(cc + 1) * 128, nch * 512:(nch + 1) * 512])

            # stage 0: wq + x(nch0) first (q matmuls), then wk, then tables+wv
            for cc in range(CCH):
                q3[(2 * cc) % 3].dma_start(
                    wqbf[:, cc * CPC:(cc + 1) * CPC], wq[cc * 128:(cc + 1) * 128, :])
                q3[(2 * cc + 1) % 3].dma_start(
                    xbf[:, cc * 512:(cc + 1) * 512], xT[cc * 128:(cc + 1) * 128, 0:512])
            # CC-core boot: dummy warmup AllGather during the load phase
            warm_in = dram.tile([128, 8], BF16, tag="warm_i", name="warm_i")
            warm_out = dram.tile([512, 8], BF16, tag="warm_o", name="warm_o")
            nc.gpsimd.collective_compute(
                "AllGather", mybir.AluOpType.bypass, replica_groups=RGROUPS,
                ins=[warm_in.opt()], outs=[warm_out.opt()])
            for cc in range(CCH):
                q3[cc % 3].dma_start(
                    wkbf[:, cc * CPC:(cc + 1) * CPC], wk[cc * 128:(cc + 1) * 128, :])
            nc.sync.dma_start(cos_sb[:], cosP[:])
            nc.gpsimd.dma_start(sin_sb[:], sinP[:])
            for cc in range(CCH):
                q3[cc % 2].dma_start(
                    wvbf[:, cc * CPC:(cc + 1) * CPC], wv[cc * 128:(cc + 1) * 128, :])
            nc.sync.dma_start(mask_bf[:], maskut[:])
            nc.sync.dma_start(smat_bf[:], smat[:])
            nc.gpsimd.memset(ones_sb[:], 1.0)
            # only the "ones" denominator columns need initialization
            vones = vsb[:].rearrange("p (h kb e) -> p h kb e", h=HPC, kb=KB)[
                :, :, :, 64:65]
            nc.gpsimd.memset(vones, 1.0)

            # ---------------- PSUM tag rotation (2 banks: avA / avB) ---------
            _rot = {"i": 0}

            def av_tag():
                t = "avA" if _rot["i"] % 2 == 0 else "avB"
                _rot["i"] += 1
                return t

            # ---------------- phase A: QKV + RoPE for one nch ----------------
            def emit_A(nch):
                nsl = slice(nch * 512, nch * 512 + 512)
                for wi, (w_sb, t_sb) in enumerate(((wqbf, qTbf), (wkbf, kTbf))):
                    for p in range(NPAIR):
                        ps_t = pav.tile([128, 512], F32, tag=av_tag(),
                                        name=f"pst{nch}_{wi}_{p}")
                        for cc in range(CCH):
                            nc.tensor.matmul(
                                ps_t[:],
                                w_sb[:, cc * CPC + p * 128: cc * CPC + (p + 1) * 128],
                                xbf[:, (nch * CCH + cc) * 512:(nch * CCH + cc + 1) * 512],
                                start=(cc == 0), stop=(cc == CCH - 1),
                            )
                        # RoPE: t_sb = q*cos + (S.T@q)*sin'; only qub reads ps_t
                        qub = ropep.tile([128, 512], BF16, tag="qub")
                        nc.scalar.copy(qub[:], ps_t[:])
                        rot = psp.tile([128, 1024], F32, tag="s",
                                       name=f"rot{nch}_{wi}_{p}")
                        nc.tensor.matmul(rot[:, 0:512], smat_bf[:], qub[:],
                                         start=True, stop=True)
                        t1 = ropep.tile([128, 512], F32, tag="t1")
                        nc.gpsimd.tensor_tensor(t1[:], qub[:], cos_sb[:, nsl], ALU.mult)
                        t2 = ropep.tile([128, 512], F32, tag="t2")
                        nc.vector.tensor_mul(t2[:], rot[:, 0:512], sin_sb[:, nsl])
                        nc.vector.tensor_add(
                            t_sb[:, p * T + nch * 512: p * T + nch * 512 + 512],
                            t1[:], t2[:])
                # v natural layout -> vsb [128, h*KB*65 + kb*65 + (0..64)]
                for tch in range(4 * nch, 4 * nch + 4):
                    ps_v = pav.tile([128, 512], F32, tag=av_tag(), name=f"psv{tch}")
                    for cc in range(CCH):
                        nc.tensor.matmul(
                            ps_v[:, 0:CPC],
                            xbf[:, (nch * CCH + cc) * 512 + (tch % 4) * 128:
                                (nch * CCH + cc) * 512 + (tch % 4) * 128 + 128],
                            wvbf[:, cc * CPC:(cc + 1) * CPC],
                            start=(cc == 0), stop=(cc == CCH - 1),
                        )
                    # one strided DVE copy: [128, (4 heads, stride KB*65), 1, 64]
                    src = ps_v[:, 0:CPC].rearrange("p (h o d) -> p h o d", h=HPC, o=1)
                    dstv = vsb[:].rearrange("p (h kb e) -> p h kb e", h=HPC, kb=KB)[
                        :, :, tch:tch + 1, 0:64]
                    nc.vector.tensor_copy(dstv, src)

            # ---------------- phase B: attention q-chunk (both pairs) --------
            # bands: qc 0/1 combined [256,512]; qc 2/3 per-pair [128,512]
            bands = {0: dram.tile([CPC, 512], BF16, tag="agin0", name="band0"),
                     1: dram.tile([CPC, 512], BF16, tag="agin1", name="band1")}
            bandsp = {(qc, p): dram.tile([128, 512], BF16, tag=f"agin{qc}_{p}",
                                         name=f"band{qc}_{p}")
                      for qc in (2, 3) for p in range(NPAIR)}
            ag_outs = {}

            def emit_flush(qc, p, avh):
                den = smp.tile([1, 1024], BF16, tag="den", name=f"den{qc}_{p}")
                nc.scalar.copy(den[:, 0:512], avh[0][64:65, :])
                nc.scalar.copy(den[:, 512:1024], avh[1][64:65, :])
                bc = psp.tile([128, 1024], F32, tag="s", name=f"bc{qc}_{p}")
                for i in range(2):
                    nc.tensor.matmul(bc[i * 64:(i + 1) * 64, 0:512], ones_sb[:],
                                     den[:, i * 512:(i + 1) * 512],
                                     start=True, stop=True, tile_position=(0, i * 64))
                recb = smp.tile([128, 512], F32, tag="recb", name=f"recb{qc}_{p}")
                nc.vector.reciprocal_approx_fast(recb[:], bc[:, 0:512])
                ob = outbp.tile([128, 512], BF16, tag="ob", name=f"ob{qc}_{p}")
                for i in range(2):
                    nc.vector.tensor_mul(ob[i * 64:(i + 1) * 64, :],
                                         avh[i][0:64, :], recb[i * 64:(i + 1) * 64, :])
                if qc >= 2:
                    nc.sync.dma_start(bandsp[(qc, p)][:], ob[:])
                    ag_out = dram.tile([4 * 128, 512], BF16, tag=f"agout{qc}_{p}",
                                       name=f"agout{qc}_{p}")
                    nc.gpsimd.collective_compute(
                        "AllGather", mybir.AluOpType.bypass,
                        replica_groups=RGROUPS,
                        ins=[bandsp[(qc, p)].opt()], outs=[ag_out.opt()])
                    ag_outs[(qc, p)] = ag_out
                else:
                    nc.sync.dma_start(bands[qc][p * 128:(p + 1) * 128, :], ob[:])
                    if p == NPAIR - 1:
                        ag_out = dram.tile([4 * CPC, 512], BF16, tag=f"agout{qc}",
                                           name=f"agout{qc}")
                        nc.gpsimd.collective_compute(
                            "AllGather", mybir.AluOpType.bypass,
                            replica_groups=RGROUPS,
                            ins=[bands[qc].opt()], outs=[ag_out.opt()])
                        ag_outs[qc] = ag_out

            def emit_B(qc, pair=None, pend=None):
                kmax = 4 * qc + 4
                if pend is None:
                    pend = []

                def drain(nleft):
                    while len(pend) > nleft:
                        pend.pop(0)()

                for p in ((0, 1) if pair is None else (pair,)):
                    avh = [pav.tile([65, 512], F32, tag=av_tag(),
                                    name=f"av{qc}_{p}_{i}") for i in range(2)]
                    for kb in range(kmax):
                        nqs = max(qc * 512, kb * 128)
                        noff = nqs - qc * 512
                        n = 512 - noff
                        diag = (nqs == kb * 128)
                        ps_s = psp.tile([128, 1024], F32, tag="s",
                                        name=f"pss{qc}_{p}_{kb}")
                        for i in range(2):
                            hs = slice(i * 64, (i + 1) * 64)
                            nc.tensor.matmul(
                                ps_s[:, i * 512: i * 512 + n],
                                kTbf[hs, p * T + kb * 128: p * T + kb * 128 + 128],
                                qTbf[hs, p * T + nqs: p * T + nqs + n],
                                start=True, stop=True,
                                tile_position=(i * 64, 0),
                            )
                        e = expp.tile([128, 1024], BF16, tag="e",
                                      name=f"e{qc}_{p}_{kb}")
                        if (qc, kb) in DVE_KBS:
                            nc.vector.tensor_scalar(
                                e[:, 0:512 + n].bitcast(I16), ps_s[:, 0:512 + n],
                                EXP_A, EXP_B, ALU.mult, ALU.add)
                        else:
                            nc.scalar.activation(e[:, 0:512 + n], ps_s[:, 0:512 + n],
                                                 AF.Exp, scale=0.125)
                        if diag:
                            ev = e[:].rearrange("p (b c) -> p b c", b=2)[:, :, 0:128]
                            mv = mask_bf[:].rearrange("p (b c) -> p b c", b=2)
                            nc.gpsimd.tensor_tensor(ev, ev, mv, ALU.mult)

                        def mk_av(p=p, kb=kb, noff=noff, n=n, e=e, avh=avh):
                            def go():
                                for i in range(2):
                                    h = 2 * p + i
                                    vbase = h * KB * 65 + kb * 65
                                    nc.tensor.matmul(
                                        avh[i][:, noff:512],
                                        vsb[:, vbase: vbase + 65],
                                        e[:, i * 512: i * 512 + n],
                                        start=(kb == 0), stop=(kb == kmax - 1),
                                    )
                                if kb == kmax - 1:
                                    emit_flush(qc, p, avh)
                            return go

                        pend.append(mk_av())
                        drain(2)
                if pair is None or pair == 1:
                    drain(0)
                return pend

            # ---------------- phase C: Wo per q-chunk (transposed output) ----
            _ag_sb = {}

            def wo_srcs(qc):
                if qc >= 2:
                    order = [0, 2, 4, 6, 1, 3, 5, 7]
                    srcs = {cc: (ag_outs[(qc, cc % 2)], (cc // 2) * 128)
                            for cc in range(CCH)}
                else:
                    order = list(range(CCH))
                    srcs = {cc: (ag_outs[qc], cc * 128) for cc in range(CCH)}
                return order, srcs

            def emit_wo_loads(qc, half=None):
                order, srcs = wo_srcs(qc)
                for j, cc in enumerate(order):
                    if half is not None and cc % 2 != half:
                        continue
                    src, row = srcs[cc]
                    t = agp.tile([128, 512], BF16, name=f"ag_{qc}_{cc}", tag="ag")
                    q3[(j % 2) * 2].dma_start(t[:], src[row:row + 128, :])
                    _ag_sb[(qc, cc)] = t

            def emit_wo_mms(qc):
                order, srcs = wo_srcs(qc)
                for mch in range(2):
                    ps_o = pav.tile([128, 512], F32, tag=av_tag(),
                                    name=f"pso{qc}_{mch}")
                    for idx, cc in enumerate(order):
                        nc.tensor.matmul(
                            ps_o[:],
                            wobf[:, cc * CPC + mch * 128: cc * CPC + (mch + 1) * 128],
                            _ag_sb[(qc, cc)][:],
                            start=(idx == 0), stop=(idx == CCH - 1),
                        )
                    osb = outbp.tile([128, 512], BF16, tag="osb")
                    nc.scalar.copy(osb[:], ps_o[:])
                    nc.sync.dma_start(
                        out[mch * 128:(mch + 1) * 128, qc * 512:(qc + 1) * 512], osb[:])

            # ---------------- schedule: A0 B0 A1 B1 A2 A3 B3 B2 -------------
            emit_A(0)
            load_x(1, [nc.sync])
            for cc in range(CCH):
                q3[cc % 2 * 2].dma_start(
                    wobf[:, cc * CPC:(cc + 1) * CPC], wo[cc * 128:(cc + 1) * 128, :])
            emit_B(0)
            emit_wo_loads(0)
            load_x(2, [nc.sync])
            emit_A(1)
            emit_B(1)
            emit_wo_loads(1)
            load_x(3, [nc.sync])
            emit_A(2)
            emit_wo_mms(0)
            emit_A(3)
            emit_wo_mms(1)
            emit_B(3)
            emit_wo_loads(3)
            emit_B(2)
            emit_wo_loads(2)
            emit_wo_mms(3)
            emit_wo_mms(2)
    return nc


def _get_nc():
    if "nc" not in _cache:
        nc = _build_nc()
        nc.finalize()
        _cache["nc"] = nc
    return _cache["nc"]


def _host_tables(freqs_cos, freqs_sin):
    cosP = np.empty((128, T), np.float32)
    sinP = np.empty((128, T), np.float32)
    for r in range(128):
        i = (r % 64) // 2
        cosP[r] = freqs_cos[:, i]
        sinP[r] = freqs_sin[:, i]
    maskut = np.tile(np.triu(np.ones((128, 128), np.float32)), (1, 2))
    smat = np.zeros((128, 128), np.float32)
    for i in range(64):
        smat[2 * i + 1, 2 * i] = -1.0   # rot[2i] = -q[2i+1]
        smat[2 * i, 2 * i + 1] = 1.0    # rot[2i+1] = +q[2i]
    return cosP, sinP, maskut, smat


def _install_trace_hooks():
    import sys, types
    try:
        import antenv.axon_hooks  # noqa: F401
        return True
    except ImportError:
        pass
    try:
        from trn_agent_boot.trn_boot import _ntff_profile_via_ctypes
        mod = types.ModuleType("antenv.axon_hooks")
        mod._hook = _ntff_profile_via_ctypes("/opt/axon/libaxon_pjrt.so")
        mod.set_axon_ntff_profile_hook = lambda h: setattr(mod, "_hook", h)
        mod.get_axon_ntff_profile_hook = lambda: mod._hook
        sys.modules["antenv.axon_hooks"] = mod
        import antenv
        antenv.axon_hooks = mod
        import concourse.bass_utils as bu
        bu.upload_artifacts = lambda tmpdir: f"file://{tmpdir}"
        return True
    except Exception:
        return False


def _bf16(a):
    return np.ascontiguousarray(a).astype(ml_dtypes.bfloat16)


def kernel(x, freqs_cos, freqs_sin, Wq, Wk, Wv, Wo, _trace=False):
    x = np.asarray(x, np.float32)
    freqs_cos = np.asarray(freqs_cos, np.float32)
    freqs_sin = np.asarray(freqs_sin, np.float32)
    Wq, Wk, Wv, Wo = (np.asarray(w, np.float32) for w in (Wq, Wk, Wv, Wo))
    cosP, sinP, maskut, smat = _host_tables(freqs_cos, freqs_sin)

    in_maps = []
    for c in range(NCORES):
        b, g = c // 4, c % 4
        sl = slice(g * CPC, (g + 1) * CPC)
        in_maps.append({
            "xT": _bf16(x[b].T),
            "wq": _bf16(Wq[:, sl]),
            "wk": _bf16(Wk[:, sl]),
            "wv": _bf16(Wv[:, sl]),
            "wo": _bf16(Wo[:, sl]),
            "cosP": _bf16(cosP), "sinP": _bf16(sinP),
            "maskut": _bf16(maskut), "smat": _bf16(smat),
        })

    nc = _get_nc()
    if _trace:
        _trace = _install_trace_hooks()
    res = run_bass_kernel_spmd(nc, in_maps, core_ids=list(range(NCORES)), trace=_trace)
    _cache["last_res"] = res

    out = np.empty((B, T, C), np.float32)
    for c in range(NCORES):
        b, g = c // 4, c % 4
        out[b][:, g * CPC:(g + 1) * CPC] = res.results[c]["out"].T.astype(np.float32)
    return out
